# revision 11
# baseline (speedup 1.0000x reference)
"""Trainium2 Bass kernel for per-position multi-head "attention across heads".

Reference math (per position r):
    Q = x @ Wq.T ; K = x @ Wk.T ; V = x @ Wv.T          (H=1024, nh=16, hd=64)
    scores[r, i, j] = (1/8) * sum_d Q[r,i,d] * K[r,j,d]   -> [nh, nh] per position
    attn = softmax(scores, axis=-1)
    out[r, i, :] = sum_j attn[r,i,j] * V[r, j, :]

Strategy (8 NeuronCores, data-parallel over the 8192 = B*L positions; each
core handles R=1024 positions):
  - x^T shard [1024,1024] and Wq^T/Wk^T/Wv^T are pre-transposed AND cast to
    bf16 on the HOST (numpy), so the device just does plain contiguous DMA
    loads of the projection operands (contraction dim on partitions) - no
    on-device casts, no input-side XBAR transposes.
  - Projections compute NATURAL-layout Q/K/V tiles [r, o] on the
    TensorEngine (PSUM fp32), evicted (DVE, cast bf16) to DRAM staging.
  - Q/K staging is [r, head, 128]: cols 0..63 = head data, cols 64..72 =
    constant "mask bias" rows (see below), cols 73..127 dead. One XBAR
    transpose per quarter yields position-major pm[d, r*16 + head]
    (partitions 0..72 live).
  - Scores for 8 positions at a time via ONE K=73 matmul:
      lhsT = K^T pm slice [73, 128], rhs = Q^T pm slice [73, 128]
      -> PSUM [(pos,j), (pos,i)].
    Contraction rows 64..72 implement the block-diagonal mask as math:
    rows 64+k (k<8) hold sqrt(C)*[pos==k] on both sides, row 72 holds
    +sqrt(C) (K side) and -sqrt(C) (Q side), so the matmul adds
    C*[pos_a==pos_b] - C to every score: off-diagonal (cross-position)
    garbage blocks get -C (C=12.625^2, scale*C ~ 19.9 => exp ~ 2e-9 ~ 0)
    while diagonal blocks are exactly unchanged. No mask multiply needed.
  - exp via ScalarE (no max subtraction: |scale*scores| <= ~4) into a
    persistent E_all buffer; ALL score matmuls run before ALL AV matmuls so
    the PE stream is dense (no per-batch PE<->ACT latency coupling).
  - V staging is UNPADDED [r, head, 64]; the AV operand Vs[(r j), d] is a
    pure reshape of it, DMA-loaded (no transpose) as [128,(b),g,d] tiles
    with a 65th column of ones (softmax denominator trick).
  - AV via matmul with lhsT = E [(pos j), (pos i)], rhs = Vs -> PSUM
    [(pos,i), d|Z] is already in NATURAL row-major layout: normalize rows
    by 1/Z on the VectorE straight out of PSUM and DMA-scatter to out.
"""

import numpy as np

import concourse.bass as bass
import concourse.mybir as mybir
import concourse.tile as tile
from concourse import bacc

F32 = mybir.dt.float32
BF16 = mybir.dt.bfloat16

B, L, H = 4, 2048, 1024
NH, HD = 16, 64
P = 128
N_CORES = 8
R = (B * L) // N_CORES          # positions per core = 1024
KC = H // P                     # contraction chunks = 8
GS = 8                          # positions per attention group
GB = 4                          # groups per PSUM-bank batch
SCALE = 1.0 / np.sqrt(HD)
RTB = 12.625                    # sqrt(C); C=159.39, SCALE*C ~ 19.9
NBIAS = 9                       # 8 one-hot rows + 1 constant row


def build_nc(r_core=R):
    RC = r_core
    RT = RC // P                # x row tiles
    NGRP = RC // GS             # attention groups
    NBATCH = NGRP // GB         # group batches
    NQ = max(1, RT // 2)        # staging quarters (2 row tiles each)
    QR = RC // NQ               # rows per quarter

    nc = bacc.Bacc(None, target_bir_lowering=False, debug=False)

    xT_d = nc.dram_tensor("xT", [H, RC], BF16, kind="ExternalInput")
    wT_d = {m: nc.dram_tensor(f"wT_{m}", [H, H], BF16, kind="ExternalInput")
            for m in "qkv"}
    bias_d = {m: nc.dram_tensor(f"bias_{m}", [P, NBIAS], BF16,
                                kind="ExternalInput") for m in "qk"}
    out = nc.dram_tensor("out", [RC, H], F32, kind="ExternalOutput")

    with tile.TileContext(nc) as tc:
        with tc.tile_pool(name="const", bufs=1) as constp, \
             tc.tile_pool(name="persist", bufs=1) as persist, \
             tc.tile_pool(name="dram", bufs=1, space="DRAM") as dram:
            bias_sb = {m: constp.tile([P, NBIAS], BF16, name=f"bias_sb_{m}")
                       for m in "qk"}
            for m in "qk":
                nc.sync.dma_start(bias_sb[m][:], bias_d[m][:])

            # per-quarter staging tensors: Tile tracks DRAM deps per-tensor,
            # so one tensor serializes quarter q+1's writes behind quarter
            # q's XBAR read (false WAR), stalling the PE every quarter.
            stag = {m: [dram.tile([QR, NH, P if m != "v" else HD], BF16,
                                  name=f"stag_{m}{q}") for q in range(NQ)]
                    for m in "qkv"}

            # persistent SBUF tensors
            xT = persist.tile([P, KC, RC], BF16)        # x^T chunks [h, kc, r]
            # position-major Q^T/K^T: pm[d, r*NH + head]; partitions 64..72
            # are the mask-bias rows, 73..127 dead.
            pm = {m: persist.tile([P, RC * NH], BF16, name=f"pm_{m}")
                  for m in "qk"}
            # Vs[(s j), b, g, d|1]: AV moving operand + ones column
            vs = persist.tile([P, NBATCH, GB, HD + 1], BF16, name="vs")
            nc.vector.memset(vs[:, :, :, HD], 1.0)

            # ---- phase 0+1: input loads, projections -> staging -> pm/vs ----
            with tc.tile_pool(name="wt", bufs=1) as wtp, \
                 tc.tile_pool(name="ev", bufs=1) as evp, \
                 tc.tile_pool(name="projps", bufs=4, space="PSUM") as projpsp:
                wT = {m: wtp.tile([P, KC, H], BF16, name=f"wT_{m}")
                      for m in "qkv"}
                # plain contiguous loads of pre-transposed operands
                # (scalar HWDGE ring; sync ring is reserved for pm XBARs).
                # xT/wq are loaded per-kc chunk so the first projection
                # matmul (which only needs kc=0 of both) unblocks after
                # ~512KB instead of ~4MB.
                for kc in range(KC):
                    nc.scalar.dma_start(
                        xT[:, kc, :], xT_d[kc * P:(kc + 1) * P, :])
                    nc.scalar.dma_start(
                        wT["q"][:, kc, :], wT_d["q"][kc * P:(kc + 1) * P, :])
                for m in "kv":
                    nc.scalar.dma_start(
                        wT[m][:], wT_d[m].rearrange("(kc p) o -> p kc o", p=P))

                NBUF = 6
                qn_bufs = [evp.tile([P, NH, P], BF16, name=f"qn{i}")
                           for i in range(NBUF)]
                vn_bufs = [evp.tile([P, NH, HD], BF16, name=f"vn{i}")
                           for i in range(NBUF)]
                for t in qn_bufs:
                    nc.vector.memset(t[:, :, HD + NBIAS:P], 0.0)
                ti = 0
                for m in "qkv":
                    for rt in range(RT):
                        if m == "v":
                            tile_buf = vn_bufs[ti % NBUF]
                        else:
                            tile_buf = qn_bufs[ti % NBUF]
                            nc.vector.tensor_copy(
                                tile_buf[:, :, HD:HD + NBIAS],
                                bias_sb[m][:, None, :]
                                .to_broadcast((P, NH, NBIAS)))
                        ti += 1
                        for oh in range(2):
                            pp = projpsp.tile([P, 512], F32)
                            for kc in range(KC):
                                nc.tensor.matmul(
                                    pp[:],
                                    xT[:, kc, rt * P:(rt + 1) * P],
                                    wT[m][:, kc, oh * 512:(oh + 1) * 512],
                                    start=(kc == 0), stop=(kc == KC - 1))
                            nc.vector.tensor_copy(
                                tile_buf[:, oh * 8:(oh + 1) * 8, 0:HD],
                                pp[:].rearrange("p (i d) -> p i d", d=HD))
                        rtq = max(1, RT // NQ)
                        qf = rt // rtq
                        ro = (rt % rtq) * P
                        # staging stores on gpsimd (SWDGE): the scalar ring
                        # is busy with input loads early on and per-engine
                        # FIFO order would delay store transfers behind them
                        nc.gpsimd.dma_start(
                            stag[m][qf][ro:ro + P], tile_buf[:])
                        # per-quarter pm XBAR / vs reshape-load
                        if (rt + 1) % rtq == 0:
                            if m == "v":
                                bs = QR // (GB * GS)
                                nc.gpsimd.dma_start(
                                    vs[:, qf * bs:(qf + 1) * bs, :, 0:HD],
                                    stag["v"][qf].rearrange(
                                        "(b g s) j d -> (s j) b g d",
                                        g=GB, s=GS))
                            else:
                                nc.sync.dma_start_transpose(
                                    pm[m][:, qf * QR * NH:(qf + 1) * QR * NH],
                                    stag[m][qf]
                                    .rearrange("r i d -> (r i) d"))

            # ---- phase 2: attention (all scores+exp, then all AV) ----
            KB = HD + NBIAS
            with tc.tile_pool(name="eall", bufs=1) as eallp, \
                 tc.tile_pool(name="sps", bufs=3, space="PSUM") as spsp, \
                 tc.tile_pool(name="avps", bufs=4, space="PSUM") as avpsp, \
                 tc.tile_pool(name="att", bufs=6) as attp:
                E_all = eallp.tile([P, NBATCH, GB, P], BF16, name="E_all")
                for b in range(NBATCH):
                    ps = spsp.tile([P, GB, P], F32)
                    for g in range(GB):
                        c0 = (b * GB + g) * GS * NH
                        nc.tensor.matmul(
                            ps[:, g, :],
                            pm["k"][0:KB, c0:c0 + GS * NH],
                            pm["q"][0:KB, c0:c0 + GS * NH],
                            start=(g == 0), stop=(g == GB - 1))
                    nc.scalar.activation(
                        E_all[:, b], ps[:], mybir.ActivationFunctionType.Exp,
                        scale=float(SCALE))
                for b in range(NBATCH):
                    pav = avpsp.tile([P, GB, HD + 1], F32)
                    for g in range(GB):
                        nc.tensor.matmul(
                            pav[:, g, :], E_all[:, b, g, :], vs[:, b, g, :],
                            start=(g == 0), stop=(g == GB - 1))
                    rz = attp.tile([P, GB], F32, tag="rz")
                    nc.vector.reciprocal(rz[:], pav[:, :, HD])
                    onr = attp.tile([P, GB, HD], F32, tag="onr")
                    nc.vector.tensor_tensor(
                        onr[:], pav[:, :, 0:HD],
                        rz[:, :, None].to_broadcast((P, GB, HD)),
                        mybir.AluOpType.mult)
                    nc.sync.dma_start(
                        out[b * GB * GS:(b + 1) * GB * GS, :]
                        .rearrange("(g s) (i d) -> (s i) g d", s=GS, d=HD),
                        onr[:])

    nc.compile()
    return nc


def _consts():
    import ml_dtypes
    bq = np.zeros((P, NBIAS), np.float32)
    bk = np.zeros((P, NBIAS), np.float32)
    for p in range(P):
        bq[p, p % GS] = RTB
        bk[p, p % GS] = RTB
    bq[:, GS] = -RTB
    bk[:, GS] = RTB
    return {"bias_q": bq.astype(ml_dtypes.bfloat16),
            "bias_k": bk.astype(ml_dtypes.bfloat16)}


_NC_CACHE = {}


def make_in_maps(x, Wq, Wk, Wv):
    import ml_dtypes
    bf = ml_dtypes.bfloat16
    xTf = np.asarray(x, np.float32).reshape(B * L, H).astype(bf).T
    wT = {m: np.ascontiguousarray(np.asarray(w, np.float32).astype(bf).T)
          for m, w in (("q", Wq), ("k", Wk), ("v", Wv))}
    consts = _consts()
    maps = []
    for c in range(N_CORES):
        m = {"xT": np.ascontiguousarray(xTf[:, c * R:(c + 1) * R]),
             "wT_q": wT["q"], "wT_k": wT["k"], "wT_v": wT["v"]}
        m.update(consts)
        maps.append(m)
    return maps


def kernel(x, Wq, Wk, Wv):
    from concourse.bass_utils import run_bass_kernel_spmd

    if "nc" not in _NC_CACHE:
        _NC_CACHE["nc"] = build_nc()
    res = run_bass_kernel_spmd(_NC_CACHE["nc"], make_in_maps(x, Wq, Wk, Wv),
                               core_ids=list(range(N_CORES)))
    outs = [r["out"] for r in res.results]
    return np.concatenate(outs, axis=0).reshape(B, L, H).astype(np.float32)


# revision 13
# speedup vs baseline: 1.0003x; 1.0003x over previous
"""Trainium2 Bass kernel for per-position multi-head "attention across heads".

Reference math (per position r):
    Q = x @ Wq.T ; K = x @ Wk.T ; V = x @ Wv.T          (H=1024, nh=16, hd=64)
    scores[r, i, j] = (1/8) * sum_d Q[r,i,d] * K[r,j,d]   -> [nh, nh] per position
    attn = softmax(scores, axis=-1)
    out[r, i, :] = sum_j attn[r,i,j] * V[r, j, :]

Strategy (8 NeuronCores, data-parallel over the 8192 = B*L positions; each
core handles R=1024 positions):
  - x^T shard [1024,1024] and Wq^T/Wk^T/Wv^T are pre-transposed AND cast to
    bf16 on the HOST (numpy), so the device just does plain contiguous DMA
    loads of the projection operands (contraction dim on partitions) - no
    on-device casts, no input-side XBAR transposes.
  - Projections compute NATURAL-layout Q/K/V tiles [r, o] on the
    TensorEngine (PSUM fp32), evicted (DVE, cast bf16) to DRAM staging.
  - Q/K staging is [r, head, 128]: cols 0..63 = head data, cols 64..72 =
    constant "mask bias" rows (see below), cols 73..127 dead. One XBAR
    transpose per quarter yields position-major pm[d, r*16 + head]
    (partitions 0..72 live).
  - Scores for 8 positions at a time via ONE K=73 matmul:
      lhsT = K^T pm slice [73, 128], rhs = Q^T pm slice [73, 128]
      -> PSUM [(pos,j), (pos,i)].
    Contraction rows 64..72 implement the block-diagonal mask as math:
    rows 64+k (k<8) hold sqrt(C)*[pos==k] on both sides, row 72 holds
    +sqrt(C) (K side) and -sqrt(C) (Q side), so the matmul adds
    C*[pos_a==pos_b] - C to every score: off-diagonal (cross-position)
    garbage blocks get -C (C=12.625^2, scale*C ~ 19.9 => exp ~ 2e-9 ~ 0)
    while diagonal blocks are exactly unchanged. No mask multiply needed.
  - exp via ScalarE (no max subtraction: |scale*scores| <= ~4) into a
    persistent E_all buffer; ALL score matmuls run before ALL AV matmuls so
    the PE stream is dense (no per-batch PE<->ACT latency coupling).
  - V staging is UNPADDED [r, head, 64]; the AV operand Vs[(r j), d] is a
    pure reshape of it, DMA-loaded (no transpose) as [128,(b),g,d] tiles
    with a 65th column of ones (softmax denominator trick).
  - AV via matmul with lhsT = E [(pos j), (pos i)], rhs = Vs -> PSUM
    [(pos,i), d|Z] is already in NATURAL row-major layout: normalize rows
    by 1/Z on the VectorE straight out of PSUM and DMA-scatter to out.
"""

import numpy as np

import concourse.bass as bass
import concourse.mybir as mybir
import concourse.tile as tile
from concourse import bacc

F32 = mybir.dt.float32
BF16 = mybir.dt.bfloat16

B, L, H = 4, 2048, 1024
NH, HD = 16, 64
P = 128
N_CORES = 8
R = (B * L) // N_CORES          # positions per core = 1024
KC = H // P                     # contraction chunks = 8
GS = 8                          # positions per attention group
GB = 4                          # groups per PSUM-bank batch
SCALE = 1.0 / np.sqrt(HD)
RTB = 12.625                    # sqrt(C); C=159.39, SCALE*C ~ 19.9
NBIAS = 9                       # 8 one-hot rows + 1 constant row


def build_nc(r_core=R):
    RC = r_core
    RT = RC // P                # x row tiles
    NGRP = RC // GS             # attention groups
    NBATCH = NGRP // GB         # group batches
    NQ = max(1, RT // 2)        # staging quarters (2 row tiles each)
    QR = RC // NQ               # rows per quarter

    nc = bacc.Bacc(None, target_bir_lowering=False, debug=False)

    xT_d = nc.dram_tensor("xT", [H, RC], BF16, kind="ExternalInput")
    wT_d = {m: nc.dram_tensor(f"wT_{m}", [H, H], BF16, kind="ExternalInput")
            for m in "qkv"}
    bias_d = {m: nc.dram_tensor(f"bias_{m}", [P, NBIAS], BF16,
                                kind="ExternalInput") for m in "qk"}
    out = nc.dram_tensor("out", [RC, H], F32, kind="ExternalOutput")

    with tile.TileContext(nc) as tc:
        with tc.tile_pool(name="const", bufs=1) as constp, \
             tc.tile_pool(name="persist", bufs=1) as persist, \
             tc.tile_pool(name="dram", bufs=1, space="DRAM") as dram:
            bias_sb = {m: constp.tile([P, NBIAS], BF16, name=f"bias_sb_{m}")
                       for m in "qk"}
            for m in "qk":
                nc.sync.dma_start(bias_sb[m][:], bias_d[m][:])

            # per-quarter staging tensors: Tile tracks DRAM deps per-tensor,
            # so one tensor serializes quarter q+1's writes behind quarter
            # q's XBAR read (false WAR), stalling the PE every quarter.
            stag = {m: [dram.tile([QR, NH, P if m != "v" else HD], BF16,
                                  name=f"stag_{m}{q}") for q in range(NQ)]
                    for m in "qkv"}

            # persistent SBUF tensors
            xT = persist.tile([P, KC, RC], BF16)        # x^T chunks [h, kc, r]
            # position-major Q^T/K^T: pm[d, r*NH + head]; partitions 64..72
            # are the mask-bias rows, 73..127 dead.
            pm = {m: persist.tile([P, RC * NH], BF16, name=f"pm_{m}")
                  for m in "qk"}
            # Vs[(s j), b, g, d|1]: AV moving operand + ones column
            vs = persist.tile([P, NBATCH, GB, HD + 1], BF16, name="vs")
            nc.vector.memset(vs[:, :, :, HD], 1.0)

            # ---- phase 0+1: input loads, projections -> staging -> pm/vs ----
            with tc.tile_pool(name="wt", bufs=1) as wtp, \
                 tc.tile_pool(name="ev", bufs=1) as evp, \
                 tc.tile_pool(name="projps", bufs=4, space="PSUM") as projpsp:
                wT = {m: wtp.tile([P, KC, H], BF16, name=f"wT_{m}")
                      for m in "qkv"}
                # plain contiguous loads of pre-transposed operands
                # (scalar HWDGE ring; sync ring is reserved for pm XBARs).
                # xT/wq are loaded per-kc chunk so the first projection
                # matmul (which only needs kc=0 of both) unblocks after
                # ~512KB instead of ~4MB.
                for kc in range(KC):
                    nc.scalar.dma_start(
                        xT[:, kc, :], xT_d[kc * P:(kc + 1) * P, :])
                    nc.scalar.dma_start(
                        wT["q"][:, kc, :], wT_d["q"][kc * P:(kc + 1) * P, :])
                # wk/wv ride the sync ring (idle until the first pm XBAR
                # ~30us in) so staging stores on the scalar ring are not
                # transfer-delayed behind them.
                for m in "kv":
                    nc.sync.dma_start(
                        wT[m][:], wT_d[m].rearrange("(kc p) o -> p kc o", p=P))

                NBUF = 6
                qn_bufs = [evp.tile([P, NH, P], BF16, name=f"qn{i}")
                           for i in range(NBUF)]
                vn_bufs = [evp.tile([P, NH, HD], BF16, name=f"vn{i}")
                           for i in range(NBUF)]
                for t in qn_bufs:
                    nc.vector.memset(t[:, :, HD + NBIAS:P], 0.0)
                ti = 0
                for m in "qkv":
                    for rt in range(RT):
                        if m == "v":
                            tile_buf = vn_bufs[ti % NBUF]
                        else:
                            tile_buf = qn_bufs[ti % NBUF]
                            nc.vector.tensor_copy(
                                tile_buf[:, :, HD:HD + NBIAS],
                                bias_sb[m][:, None, :]
                                .to_broadcast((P, NH, NBIAS)))
                        ti += 1
                        for oh in range(2):
                            pp = projpsp.tile([P, 512], F32)
                            for kc in range(KC):
                                nc.tensor.matmul(
                                    pp[:],
                                    xT[:, kc, rt * P:(rt + 1) * P],
                                    wT[m][:, kc, oh * 512:(oh + 1) * 512],
                                    start=(kc == 0), stop=(kc == KC - 1))
                            nc.vector.tensor_copy(
                                tile_buf[:, oh * 8:(oh + 1) * 8, 0:HD],
                                pp[:].rearrange("p (i d) -> p i d", d=HD))
                        rtq = max(1, RT // NQ)
                        qf = rt // rtq
                        ro = (rt % rtq) * P
                        nc.scalar.dma_start(
                            stag[m][qf][ro:ro + P], tile_buf[:])
                        # per-quarter pm XBAR / vs reshape-load
                        if (rt + 1) % rtq == 0:
                            if m == "v":
                                bs = QR // (GB * GS)
                                nc.gpsimd.dma_start(
                                    vs[:, qf * bs:(qf + 1) * bs, :, 0:HD],
                                    stag["v"][qf].rearrange(
                                        "(b g s) j d -> (s j) b g d",
                                        g=GB, s=GS))
                            else:
                                nc.sync.dma_start_transpose(
                                    pm[m][:, qf * QR * NH:(qf + 1) * QR * NH],
                                    stag[m][qf]
                                    .rearrange("r i d -> (r i) d"))

            # ---- phase 2: attention (all scores+exp, then all AV) ----
            KB = HD + NBIAS
            with tc.tile_pool(name="eall", bufs=1) as eallp, \
                 tc.tile_pool(name="sps", bufs=3, space="PSUM") as spsp, \
                 tc.tile_pool(name="avps", bufs=4, space="PSUM") as avpsp, \
                 tc.tile_pool(name="att", bufs=6) as attp:
                E_all = eallp.tile([P, NBATCH, GB, P], BF16, name="E_all")
                for b in range(NBATCH):
                    ps = spsp.tile([P, GB, P], F32)
                    for g in range(GB):
                        c0 = (b * GB + g) * GS * NH
                        nc.tensor.matmul(
                            ps[:, g, :],
                            pm["k"][0:KB, c0:c0 + GS * NH],
                            pm["q"][0:KB, c0:c0 + GS * NH],
                            start=(g == 0), stop=(g == GB - 1))
                    nc.scalar.activation(
                        E_all[:, b], ps[:], mybir.ActivationFunctionType.Exp,
                        scale=float(SCALE))
                for b in range(NBATCH):
                    pav = avpsp.tile([P, GB, HD + 1], F32)
                    for g in range(GB):
                        nc.tensor.matmul(
                            pav[:, g, :], E_all[:, b, g, :], vs[:, b, g, :],
                            start=(g == 0), stop=(g == GB - 1))
                    rz = attp.tile([P, GB], F32, tag="rz")
                    nc.vector.reciprocal(rz[:], pav[:, :, HD])
                    onr = attp.tile([P, GB, HD], F32, tag="onr")
                    nc.vector.tensor_tensor(
                        onr[:], pav[:, :, 0:HD],
                        rz[:, :, None].to_broadcast((P, GB, HD)),
                        mybir.AluOpType.mult)
                    nc.sync.dma_start(
                        out[b * GB * GS:(b + 1) * GB * GS, :]
                        .rearrange("(g s) (i d) -> (s i) g d", s=GS, d=HD),
                        onr[:])

    nc.compile()
    return nc


def _consts():
    import ml_dtypes
    bq = np.zeros((P, NBIAS), np.float32)
    bk = np.zeros((P, NBIAS), np.float32)
    for p in range(P):
        bq[p, p % GS] = RTB
        bk[p, p % GS] = RTB
    bq[:, GS] = -RTB
    bk[:, GS] = RTB
    return {"bias_q": bq.astype(ml_dtypes.bfloat16),
            "bias_k": bk.astype(ml_dtypes.bfloat16)}


_NC_CACHE = {}


def make_in_maps(x, Wq, Wk, Wv):
    import ml_dtypes
    bf = ml_dtypes.bfloat16
    xTf = np.asarray(x, np.float32).reshape(B * L, H).astype(bf).T
    wT = {m: np.ascontiguousarray(np.asarray(w, np.float32).astype(bf).T)
          for m, w in (("q", Wq), ("k", Wk), ("v", Wv))}
    consts = _consts()
    maps = []
    for c in range(N_CORES):
        m = {"xT": np.ascontiguousarray(xTf[:, c * R:(c + 1) * R]),
             "wT_q": wT["q"], "wT_k": wT["k"], "wT_v": wT["v"]}
        m.update(consts)
        maps.append(m)
    return maps


def kernel(x, Wq, Wk, Wv):
    from concourse.bass_utils import run_bass_kernel_spmd

    if "nc" not in _NC_CACHE:
        _NC_CACHE["nc"] = build_nc()
    res = run_bass_kernel_spmd(_NC_CACHE["nc"], make_in_maps(x, Wq, Wk, Wv),
                               core_ids=list(range(N_CORES)))
    outs = [r["out"] for r in res.results]
    return np.concatenate(outs, axis=0).reshape(B, L, H).astype(np.float32)


# revision 14
# speedup vs baseline: 1.0356x; 1.0353x over previous
"""Trainium2 Bass kernel for per-position multi-head "attention across heads".

Reference math (per position r):
    Q = x @ Wq.T ; K = x @ Wk.T ; V = x @ Wv.T          (H=1024, nh=16, hd=64)
    scores[r, i, j] = (1/8) * sum_d Q[r,i,d] * K[r,j,d]   -> [nh, nh] per position
    attn = softmax(scores, axis=-1)
    out[r, i, :] = sum_j attn[r,i,j] * V[r, j, :]

Strategy (8 NeuronCores, data-parallel over the 8192 = B*L positions; each
core handles R=1024 positions):
  - x^T shard [1024,1024] and Wq^T/Wk^T/Wv^T are pre-transposed AND cast to
    bf16 on the HOST (numpy), so the device just does plain contiguous DMA
    loads of the projection operands (contraction dim on partitions) - no
    on-device casts, no input-side XBAR transposes.
  - Projections compute NATURAL-layout Q/K/V tiles [r, o] on the
    TensorEngine (PSUM fp32), evicted (DVE, cast bf16) to DRAM staging.
  - Q/K staging is [r, head, 128]: cols 0..63 = head data, cols 64..72 =
    constant "mask bias" rows (see below), cols 73..127 dead. One XBAR
    transpose per quarter yields position-major pm[d, r*16 + head]
    (partitions 0..72 live).
  - Scores for 8 positions at a time via ONE K=73 matmul:
      lhsT = K^T pm slice [73, 128], rhs = Q^T pm slice [73, 128]
      -> PSUM [(pos,j), (pos,i)].
    Contraction rows 64..72 implement the block-diagonal mask as math:
    rows 64+k (k<8) hold sqrt(C)*[pos==k] on both sides, row 72 holds
    +sqrt(C) (K side) and -sqrt(C) (Q side), so the matmul adds
    C*[pos_a==pos_b] - C to every score: off-diagonal (cross-position)
    garbage blocks get -C (C=12.625^2, scale*C ~ 19.9 => exp ~ 2e-9 ~ 0)
    while diagonal blocks are exactly unchanged. No mask multiply needed.
  - exp via ScalarE (no max subtraction: |scale*scores| <= ~4) into a
    persistent E_all buffer; ALL score matmuls run before ALL AV matmuls so
    the PE stream is dense (no per-batch PE<->ACT latency coupling).
  - V staging is UNPADDED [r, head, 64]; the AV operand Vs[(r j), d] is a
    pure reshape of it, DMA-loaded (no transpose) as [128,(b),g,d] tiles
    with a 65th column of ones (softmax denominator trick).
  - AV via matmul with lhsT = E [(pos j), (pos i)], rhs = Vs -> PSUM
    [(pos,i), d|Z] is already in NATURAL row-major layout: normalize rows
    by 1/Z on the VectorE straight out of PSUM and DMA-scatter to out.
"""

import numpy as np

import concourse.bass as bass
import concourse.mybir as mybir
import concourse.tile as tile
from concourse import bacc

F32 = mybir.dt.float32
BF16 = mybir.dt.bfloat16

B, L, H = 4, 2048, 1024
NH, HD = 16, 64
P = 128
N_CORES = 8
R = (B * L) // N_CORES          # positions per core = 1024
KC = H // P                     # contraction chunks = 8
GS = 8                          # positions per attention group
GB = 4                          # groups per PSUM-bank batch
SCALE = 1.0 / np.sqrt(HD)
RTB = 12.625                    # sqrt(C); C=159.39, SCALE*C ~ 19.9
NBIAS = 9                       # 8 one-hot rows + 1 constant row


def build_nc(r_core=R):
    RC = r_core
    RT = RC // P                # x row tiles
    NGRP = RC // GS             # attention groups
    NBATCH = NGRP // GB         # group batches
    NQ = max(1, RT // 2)        # staging quarters (2 row tiles each)
    QR = RC // NQ               # rows per quarter

    nc = bacc.Bacc(None, target_bir_lowering=False, debug=False)

    xT_d = nc.dram_tensor("xT", [H, RC], BF16, kind="ExternalInput")
    wT_d = {m: nc.dram_tensor(f"wT_{m}", [H, H], BF16, kind="ExternalInput")
            for m in "qkv"}
    bias_d = {m: nc.dram_tensor(f"bias_{m}", [P, NBIAS], BF16,
                                kind="ExternalInput") for m in "qk"}
    out = nc.dram_tensor("out", [RC, H], F32, kind="ExternalOutput")

    with tile.TileContext(nc) as tc:
        with tc.tile_pool(name="const", bufs=1) as constp, \
             tc.tile_pool(name="persist", bufs=1) as persist, \
             tc.tile_pool(name="dram", bufs=1, space="DRAM") as dram:
            bias_sb = {m: constp.tile([P, NBIAS], BF16, name=f"bias_sb_{m}")
                       for m in "qk"}
            for m in "qk":
                nc.sync.dma_start(bias_sb[m][:], bias_d[m][:])

            # per-quarter staging tensors: Tile tracks DRAM deps per-tensor,
            # so one tensor serializes quarter q+1's writes behind quarter
            # q's XBAR read (false WAR), stalling the PE every quarter.
            stag = {m: [dram.tile([QR, NH, P if m != "v" else HD], BF16,
                                  name=f"stag_{m}{q}") for q in range(NQ)]
                    for m in "qkv"}

            # persistent SBUF tensors
            xT = persist.tile([P, KC, RC], BF16)        # x^T chunks [h, kc, r]
            # position-major Q^T/K^T: pm[d, r*NH + head]; partitions 64..72
            # are the mask-bias rows, 73..127 dead.
            pm = {m: persist.tile([P, RC * NH], BF16, name=f"pm_{m}")
                  for m in "qk"}
            # Vs[(s j), b, g, d|1]: AV moving operand + ones column
            vs = persist.tile([P, NBATCH, GB, HD + 1], BF16, name="vs")
            nc.vector.memset(vs[:, :, :, HD], 1.0)

            # ---- phase 0+1: input loads, projections -> staging -> pm/vs ----
            with tc.tile_pool(name="wt", bufs=1) as wtp, \
                 tc.tile_pool(name="ev", bufs=1) as evp, \
                 tc.tile_pool(name="projps", bufs=4, space="PSUM") as projpsp:
                wT = {m: wtp.tile([P, KC, H], BF16, name=f"wT_{m}")
                      for m in "qkv"}
                # plain contiguous loads of pre-transposed operands
                # (scalar HWDGE ring; sync ring is reserved for pm XBARs).
                # xT/wq are loaded per-kc chunk so the first projection
                # matmul (which only needs kc=0 of both) unblocks after
                # ~512KB instead of ~4MB.
                for kc in range(KC):
                    nc.scalar.dma_start(
                        xT[:, kc, :], xT_d[kc * P:(kc + 1) * P, :])
                    nc.scalar.dma_start(
                        wT["q"][:, kc, :], wT_d["q"][kc * P:(kc + 1) * P, :])
                # wk/wv ride the sync ring (idle until the first pm XBAR
                # ~30us in) so staging stores on the scalar ring are not
                # transfer-delayed behind them.
                for m in "kv":
                    nc.sync.dma_start(
                        wT[m][:], wT_d[m].rearrange("(kc p) o -> p kc o", p=P))

                NBUF = 6
                qn_bufs = [evp.tile([P, NH, P], BF16, name=f"qn{i}")
                           for i in range(NBUF)]
                vn_bufs = [evp.tile([P, NH, HD], BF16, name=f"vn{i}")
                           for i in range(NBUF)]
                for t in qn_bufs:
                    nc.vector.memset(t[:, :, HD + NBIAS:P], 0.0)
                ti = 0
                for m in "qkv":
                    for rt in range(RT):
                        if m == "v":
                            tile_buf = vn_bufs[ti % NBUF]
                        else:
                            tile_buf = qn_bufs[ti % NBUF]
                            nc.vector.tensor_copy(
                                tile_buf[:, :, HD:HD + NBIAS],
                                bias_sb[m][:, None, :]
                                .to_broadcast((P, NH, NBIAS)))
                        ti += 1
                        for oh in range(2):
                            pp = projpsp.tile([P, 512], F32)
                            for kc in range(KC):
                                nc.tensor.matmul(
                                    pp[:],
                                    xT[:, kc, rt * P:(rt + 1) * P],
                                    wT[m][:, kc, oh * 512:(oh + 1) * 512],
                                    start=(kc == 0), stop=(kc == KC - 1))
                            nc.vector.tensor_copy(
                                tile_buf[:, oh * 8:(oh + 1) * 8, 0:HD],
                                pp[:].rearrange("p (i d) -> p i d", d=HD))
                        rtq = max(1, RT // NQ)
                        qf = rt // rtq
                        ro = (rt % rtq) * P
                        nc.scalar.dma_start(
                            stag[m][qf][ro:ro + P], tile_buf[:])
                        # per-quarter pm XBAR / vs reshape-load
                        if (rt + 1) % rtq == 0:
                            if m == "v":
                                bs = QR // (GB * GS)
                                nc.gpsimd.dma_start(
                                    vs[:, qf * bs:(qf + 1) * bs, :, 0:HD],
                                    stag["v"][qf].rearrange(
                                        "(b g s) j d -> (s j) b g d",
                                        g=GB, s=GS))
                            else:
                                nc.sync.dma_start_transpose(
                                    pm[m][:, qf * QR * NH:(qf + 1) * QR * NH],
                                    stag[m][qf]
                                    .rearrange("r i d -> (r i) d"))

            # ---- phase 2: attention (all scores+exp, then all AV) ----
            # score contraction padded to 128 partitions (rows 73..127 are
            # zeros from the staging memset): NumWeights==128 enables FWL
            # on the LDWEIGHTS and keeps the PE activity monitor busier.
            KB = P
            with tc.tile_pool(name="eall", bufs=1) as eallp, \
                 tc.tile_pool(name="sps", bufs=3, space="PSUM") as spsp, \
                 tc.tile_pool(name="avps", bufs=4, space="PSUM") as avpsp, \
                 tc.tile_pool(name="att", bufs=6) as attp:
                E_all = eallp.tile([P, NBATCH, GB, P], BF16, name="E_all")
                for b in range(NBATCH):
                    ps = spsp.tile([P, GB, P], F32)
                    for g in range(GB):
                        c0 = (b * GB + g) * GS * NH
                        nc.tensor.matmul(
                            ps[:, g, :],
                            pm["k"][0:KB, c0:c0 + GS * NH],
                            pm["q"][0:KB, c0:c0 + GS * NH],
                            start=(g == 0), stop=(g == GB - 1))
                    nc.scalar.activation(
                        E_all[:, b], ps[:], mybir.ActivationFunctionType.Exp,
                        scale=float(SCALE))
                for b in range(NBATCH):
                    pav = avpsp.tile([P, GB, HD + 1], F32)
                    for g in range(GB):
                        nc.tensor.matmul(
                            pav[:, g, :], E_all[:, b, g, :], vs[:, b, g, :],
                            start=(g == 0), stop=(g == GB - 1))
                    rz = attp.tile([P, GB], F32, tag="rz")
                    nc.vector.reciprocal(rz[:], pav[:, :, HD])
                    onr = attp.tile([P, GB, HD], F32, tag="onr")
                    nc.vector.tensor_tensor(
                        onr[:], pav[:, :, 0:HD],
                        rz[:, :, None].to_broadcast((P, GB, HD)),
                        mybir.AluOpType.mult)
                    nc.sync.dma_start(
                        out[b * GB * GS:(b + 1) * GB * GS, :]
                        .rearrange("(g s) (i d) -> (s i) g d", s=GS, d=HD),
                        onr[:])

    nc.compile()
    return nc


def _consts():
    import ml_dtypes
    bq = np.zeros((P, NBIAS), np.float32)
    bk = np.zeros((P, NBIAS), np.float32)
    for p in range(P):
        bq[p, p % GS] = RTB
        bk[p, p % GS] = RTB
    bq[:, GS] = -RTB
    bk[:, GS] = RTB
    return {"bias_q": bq.astype(ml_dtypes.bfloat16),
            "bias_k": bk.astype(ml_dtypes.bfloat16)}


_NC_CACHE = {}


def make_in_maps(x, Wq, Wk, Wv):
    import ml_dtypes
    bf = ml_dtypes.bfloat16
    xTf = np.asarray(x, np.float32).reshape(B * L, H).astype(bf).T
    wT = {m: np.ascontiguousarray(np.asarray(w, np.float32).astype(bf).T)
          for m, w in (("q", Wq), ("k", Wk), ("v", Wv))}
    consts = _consts()
    maps = []
    for c in range(N_CORES):
        m = {"xT": np.ascontiguousarray(xTf[:, c * R:(c + 1) * R]),
             "wT_q": wT["q"], "wT_k": wT["k"], "wT_v": wT["v"]}
        m.update(consts)
        maps.append(m)
    return maps


def kernel(x, Wq, Wk, Wv):
    from concourse.bass_utils import run_bass_kernel_spmd

    if "nc" not in _NC_CACHE:
        _NC_CACHE["nc"] = build_nc()
    res = run_bass_kernel_spmd(_NC_CACHE["nc"], make_in_maps(x, Wq, Wk, Wv),
                               core_ids=list(range(N_CORES)))
    outs = [r["out"] for r in res.results]
    return np.concatenate(outs, axis=0).reshape(B, L, H).astype(np.float32)
